# revision 5
# baseline (speedup 1.0000x reference)
"""RNNT joint log_softmax kernel for Trainium2 (Bass/Tile), 8-core SPMD.

out[b,t,u,v] = log_softmax(f[b,t,v] + g[b,u,v], axis=v)

Sharding: 8 shards over (b, t-half): core i handles b=i//2, t in [128*(i%2), ...).
Per-core trick: exp(f+g) = exp(f)*exp(g), so every (t,u) logsumexp comes from one
small matmul S = exp(g) @ exp(f)^T contracted over v, then lse = ln(S).
Main loop per t: PE rank-1 broadcast of f_t (bf16 hi/lo split, exact to ~2^-16)
into PSUM, ACT adds -lse (per-partition bias), DVE adds G, DMA out.
"""

import numpy as np

B, T, U, V = 4, 256, 128, 1024
TSH = 128  # t-shard per core
NCORES = 8

_nc_cache = {}


def _build(tag="main"):
    if tag in _nc_cache:
        return _nc_cache[tag]
    from contextlib import ExitStack

    import concourse.bacc as bacc
    import concourse.tile as tile
    from concourse import mybir

    f32 = mybir.dt.float32
    bf16 = mybir.dt.bfloat16
    AF = mybir.ActivationFunctionType

    nc = bacc.Bacc("TRN2", debug=False, num_devices=NCORES)
    f_d = nc.dram_tensor("f_sh", [TSH, V], f32, kind="ExternalInput").ap()
    g_d = nc.dram_tensor("g_sh", [U, V], f32, kind="ExternalInput").ap()
    eye_d = nc.dram_tensor("eye", [128, 128], f32, kind="ExternalInput").ap()
    out_d = nc.dram_tensor("out_sh", [TSH, U, V], f32, kind="ExternalOutput").ap()

    with tile.TileContext(nc) as tc, ExitStack() as ctx:
        const_pool = ctx.enter_context(tc.tile_pool(name="const", bufs=1))
        out_pool = ctx.enter_context(tc.tile_pool(name="out", bufs=3))

        F = const_pool.tile([128, V], f32)
        G = const_pool.tile([128, V], f32)
        eye = const_pool.tile([128, 128], f32)
        nc.sync.dma_start(F[:], f_d[:])
        nc.sync.dma_start(G[:], g_d[:])
        nc.sync.dma_start(eye[:], eye_d[:])

        eye_bf = const_pool.tile([128, 128], bf16)
        nc.vector.tensor_copy(eye_bf[:], eye[:])

        # --- preamble: transposed exp tiles + S matmul -> -lse[u, t] ---
        EfT = const_pool.tile([128, V], f32)  # chunk c at [:, 128c:128c+128]
        EgT = const_pool.tile([128, V], f32)
        lseT = const_pool.tile([128, 128], f32)
        neg_lseT = const_pool.tile([128, 128], f32)
        with tc.tile_pool(name="psum_pre", bufs=2, space="PSUM") as pre_psum, \
             tc.tile_pool(name="psum_s", bufs=1, space="PSUM") as s_pool:
            for src, dst in ((F, EfT), (G, EgT)):
                for c in range(8):
                    tp = pre_psum.tile([128, 128], f32, tag="tp")
                    nc.tensor.transpose(tp[:], src[:, 128 * c:128 * (c + 1)], eye[:])
                    nc.scalar.activation(dst[:, 128 * c:128 * (c + 1)], tp[:], AF.Exp)
            s_ps = s_pool.tile([128, 128], f32)
            for c in range(8):
                nc.tensor.matmul(
                    s_ps[:],
                    EgT[:, 128 * c:128 * (c + 1)],
                    EfT[:, 128 * c:128 * (c + 1)],
                    start=(c == 0),
                    stop=(c == 7),
                )
            nc.scalar.activation(lseT[:], s_ps[:], AF.Ln)
        nc.scalar.mul(neg_lseT[:], lseT[:], -1.0)

        # --- bf16 hi/lo split of F for exact-ish PE broadcast ---
        F_hi = const_pool.tile([128, V], bf16)
        F_hi32 = const_pool.tile([128, V], f32)
        F_lo32 = const_pool.tile([128, V], f32)
        F_lo = const_pool.tile([128, V], bf16)
        nc.vector.tensor_copy(F_hi[:], F[:])
        nc.vector.tensor_copy(F_hi32[:], F_hi[:])
        nc.vector.tensor_sub(F_lo32[:], F[:], F_hi32[:])
        nc.vector.tensor_copy(F_lo[:], F_lo32[:])

        # --- main loop over t, grouped GT t's per output DMA (2MB writes) ---
        GT = 4
        with tc.tile_pool(name="psum_b", bufs=3, space="PSUM") as psum_b:
            for tg in range(TSH // GT):
                stage = out_pool.tile([128, GT, V], f32)
                for j in range(GT):
                    t = tg * GT + j
                    pb = psum_b.tile([128, V], f32)
                    # one-hot column t of eye as stationary operand: selects
                    # row t of F_hi/F_lo (broadcast over all output partitions)
                    onehot = eye_bf[:, t:t + 1].broadcast_to([128, 128])
                    for c2 in range(2):
                        sl = slice(512 * c2, 512 * (c2 + 1))
                        nc.tensor.matmul(
                            pb[:, sl], onehot, F_hi[:, sl],
                            start=True, stop=False,
                        )
                        nc.tensor.matmul(
                            pb[:, sl], onehot, F_lo[:, sl],
                            start=False, stop=True,
                        )
                    # pb += -lse[:, t]  (per-partition scalar bias on ACT)
                    nc.scalar.activation(
                        pb[:], pb[:], AF.Identity, bias=neg_lseT[:, t:t + 1]
                    )
                    nc.vector.tensor_add(stage[:, j, :], G[:], pb[:])
                nc.sync.dma_start(
                    out_d[tg * GT:(tg + 1) * GT].rearrange("t u v -> u t v"),
                    stage[:],
                )

    nc.compile()
    _nc_cache[tag] = nc
    return nc


def _in_maps(f, g):
    eye = np.eye(128, dtype=np.float32)
    maps = []
    for i in range(NCORES):
        b, h = divmod(i, 2)
        maps.append({
            "f_sh": np.ascontiguousarray(f[b, h * TSH:(h + 1) * TSH]),
            "g_sh": np.ascontiguousarray(g[b]),
            "eye": eye,
        })
    return maps


def _gather(results):
    out = np.empty((B, T, U, V), np.float32)
    for i in range(NCORES):
        b, h = divmod(i, 2)
        out[b, h * TSH:(h + 1) * TSH] = results[i]["out_sh"]
    return out


def kernel(**inputs):
    from concourse.bass_utils import run_bass_kernel_spmd

    f = np.asarray(inputs["f"], np.float32)
    g = np.asarray(inputs["g"], np.float32)
    nc = _build()
    res = run_bass_kernel_spmd(nc, _in_maps(f, g), core_ids=list(range(NCORES)))
    return _gather(res.results)
